# revision 13
# baseline (speedup 1.0000x reference)
"""Multi-head attention (bsz=2, seq=2048, hidden=1024, heads=16) on 8 TRN2 cores.

Sharding: core c = 4*b + g handles batch b and heads [4g, 4g+4).
Each core computes Q/K/V projections for its 4 heads, causal softmax
attention, and a partial output projection over its 256 features; the host
sums the 4 per-batch partials.

All matmuls run in float32r (1 cycle/row for N>=256, ~1e-4 rel err).
Scores are computed transposed (S^T: k on partitions, q on free dim) so the
probabilities can feed P@V directly as the moving operand. V is augmented
with 64 all-ones columns so the PV matmul also produces the softmax
denominator replicated across 64 partitions (cheap 64-lane reciprocal, no
partition broadcast needed). Scores stay in +-2.5 so exp() needs no
max-subtraction; causal masking is a 0/1 multiply on the probabilities.
Phases are emitted interleaved per 512-query chunk to keep the PE densely
busy (HAM clock gate) and overlap ACT/PE/DMA.
"""

import sys

sys.path.insert(0, "/opt/trn_rl_repo")

from contextlib import ExitStack

import numpy as np

import concourse.tile as tile
from concourse import bacc, bass_utils, mybir

B, S, H = 2, 2048, 1024
NHC = 4  # heads per core
HD = 64  # head dim
F = NHC * HD  # features per core (256)
N_CORES = 8
QC = 512  # query-chunk width
KB = 128  # key-block size
G = 2  # k-blocks per exp batch
SCALE = 1.0 / 8.0  # 1/sqrt(HD)

F32 = mybir.dt.float32
F32R = mybir.dt.float32r
EXP = mybir.ActivationFunctionType.Exp

_CACHE = {}


def _emit(tc):
    nc = tc.nc
    xT_d = nc.dram_tensor("xT", [H, S], F32R, kind="ExternalInput").ap()
    wqT_d = nc.dram_tensor("wqT", [H, F], F32R, kind="ExternalInput").ap()
    wkT_d = nc.dram_tensor("wkT", [H, F], F32R, kind="ExternalInput").ap()
    wvT_d = nc.dram_tensor("wvT", [H, F], F32R, kind="ExternalInput").ap()
    woT_d = nc.dram_tensor("woT", [F, H], F32R, kind="ExternalInput").ap()
    mtri_d = nc.dram_tensor("mtri", [KB, KB], F32R, kind="ExternalInput").ap()
    out_d = nc.dram_tensor("out", [S, H], F32, kind="ExternalOutput").ap()

    ctx = tc._emit_ctx
    const = ctx.enter_context(tc.tile_pool(name="const", bufs=1))
    persist = ctx.enter_context(tc.tile_pool(name="persist", bufs=1))
    xpool = ctx.enter_context(tc.tile_pool(name="xc", bufs=16))
    pt_pool = ctx.enter_context(tc.tile_pool(name="pt", bufs=6))
    recip_pool = ctx.enter_context(tc.tile_pool(name="recip", bufs=2))
    ostage = ctx.enter_context(tc.tile_pool(name="ostage", bufs=3))
    ps_st = ctx.enter_context(tc.tile_pool(name="psst", bufs=6, space="PSUM"))
    ps_ot = ctx.enter_context(tc.tile_pool(name="psot", bufs=2, space="PSUM"))

    # ---- weights (loaded first: the first matmuls need them) ----
    wq = [persist.tile([KB, F], F32R, tag=f"wq{i}", name=f"wq{i}") for i in range(8)]
    wk = [persist.tile([KB, F], F32R, tag=f"wk{i}", name=f"wk{i}") for i in range(8)]
    wv = [persist.tile([KB, F], F32R, tag=f"wv{i}", name=f"wv{i}") for i in range(8)]
    wo = [persist.tile([KB, H], F32R, tag=f"wo{i}", name=f"wo{i}") for i in range(2)]
    for i in range(8):
        nc.sync.dma_start(wq[i][:], wqT_d[i * KB : (i + 1) * KB, :])
        nc.sync.dma_start(wk[i][:], wkT_d[i * KB : (i + 1) * KB, :])
    # causal triangle mask (0/1), f32r
    mtri = const.tile([KB, KB], F32R, tag="mtri")
    nc.sync.dma_start(mtri[:], mtri_d[:])
    ones_f32 = const.tile([KB, NHC * HD], F32, tag="ones32")
    nc.vector.memset(ones_f32[:], 1.0)
    zeros_f32 = const.tile([KB, KB], F32, tag="zeros32")
    nc.vector.memset(zeros_f32[:], 0.0)
    zeros_r = const.tile([KB, KB], F32R, tag="zerosr")
    nc.vector.tensor_copy(zeros_r[:], zeros_f32[:])
    for i in range(8):
        nc.sync.dma_start(wv[i][:], wvT_d[i * KB : (i + 1) * KB, :])
    for i in range(2):
        nc.sync.dma_start(wo[i][:], woT_d[i * KB : (i + 1) * KB, :])

    qts = [persist.tile([KB, S], F32R, tag=f"qt{i}", name=f"qt{i}") for i in range(2)]
    kts = [persist.tile([KB, S], F32R, tag=f"kt{i}", name=f"kt{i}") for i in range(2)]
    # V_aug per k-block: [V_h (64) | ones (64)] per head -> [128, 512]
    vts = [
        persist.tile([KB, NHC * 2 * HD], F32R, tag=f"vt{i}", name=f"vt{i}")
        for i in range(16)
    ]
    ats = [persist.tile([KB, S], F32R, tag=f"at{i}", name=f"at{i}") for i in range(2)]

    for jq in range(4):
        q0 = jq * QC
        nkb = 4 * jq + 4

        # ---- xT chunks for this q-range ----
        xc = []
        for hc in range(8):
            t = xpool.tile([KB, QC], F32R, tag="xc", name=f"xc{jq}_{hc}")
            nc.sync.dma_start(t[:], xT_d[hc * KB : (hc + 1) * KB, q0 : q0 + QC])
            xc.append(t)

        # ---- Q^T / K^T projection for this chunk ----
        for w, dst in ((wq, qts), (wk, kts)):
            for fc in range(2):
                ps = ps_st.tile([KB, QC], F32, tag="st", name=f"pp{jq}_{fc}")
                for hc in range(8):
                    nc.tensor.matmul(
                        ps[:],
                        w[hc][:, fc * KB : (fc + 1) * KB],
                        xc[hc][:],
                        start=(hc == 0),
                        stop=(hc == 7),
                    )
                nc.vector.tensor_copy(dst[fc][:, q0 : q0 + QC], ps[:])

        # ---- V projection for k-blocks of this chunk ----
        for sub in range(4):
            rc = 4 * jq + sub
            psv = ps_st.tile([KB, F], F32, tag="st", name=f"pv{rc}")
            for hc in range(8):
                nc.tensor.matmul(
                    psv[:],
                    xc[hc][:, sub * KB : (sub + 1) * KB],
                    wv[hc][:],
                    start=(hc == 0),
                    stop=(hc == 7),
                )
            v_heads = vts[rc][:].rearrange("p (h d) -> p h d", h=NHC)
            nc.vector.tensor_copy(
                v_heads[:, :, 0:HD], psv[:].rearrange("p (h d) -> p h d", h=NHC)
            )
            nc.vector.tensor_copy(
                v_heads[:, :, HD : 2 * HD],
                ones_f32[:, :].rearrange("p (h d) -> p h d", h=NHC),
            )

        # ---- attention for all 4 heads on this q-chunk ----
        for h in range(NHC):
            t, po_ = h // 2, (h % 2) * HD
            po = ps_ot.tile([KB, QC], F32, tag="ot", name=f"po{jq}_{h}")
            for ik in range(nkb):
                r = ik - 4 * jq  # >= 0 on the causal diagonal region
                # column offsets: QK/PV skip fully-masked leading columns but
                # keep N >= 256 so f32r streams at full rate
                qk = 0 if r <= 0 else min(r, 2) * KB
                ex = 0 if r <= 0 else r * KB
                st = ps_st.tile([KB, QC], F32, tag="st", name=f"st{jq}_{h}_{ik}")
                nc.tensor.matmul(
                    st[:, qk:QC],
                    kts[t][po_ : po_ + HD, ik * KB : (ik + 1) * KB],
                    qts[t][po_ : po_ + HD, q0 + qk : q0 + QC],
                    start=True,
                    stop=True,
                )
                pt = pt_pool.tile([KB, QC], F32R, tag="pt", name=f"pt{jq}_{h}_{ik}")
                nc.scalar.activation(pt[:, ex:QC], st[:, ex:QC], EXP, scale=SCALE)
                if r >= 0:  # causal triangle mask on the diagonal block
                    tri = pt[:, r * KB : (r + 1) * KB]
                    nc.vector.tensor_mul(tri, tri, mtri[:])
                if r == 3:  # PV covers [256:512] but only [384:512] is live
                    nc.vector.tensor_copy(pt[:, 2 * KB : 3 * KB], zeros_r[:])
                nc.tensor.matmul(
                    po[:, qk:QC],
                    vts[ik][:, h * 2 * HD : (h + 1) * 2 * HD],
                    pt[:, qk:QC],
                    start=(ik == 0),
                    stop=(ik == nkb - 1),
                )
            # normalize: rows 64:128 of po hold the denominator (64 copies)
            recip = recip_pool.tile([HD, QC], F32R, tag="recip", name=f"rc{jq}_{h}")
            with nc.allow_low_precision(reason="f32r softmax denom"):
                nc.vector.reciprocal(recip[:], po[HD : 2 * HD, :])
            nc.vector.tensor_mul(
                ats[t][po_ : po_ + HD, q0 : q0 + QC], po[0:HD, :], recip[:]
            )

        # ---- output projection for the q-blocks of this chunk ----
        for sub in range(4):
            qb = 4 * jq + sub
            for oc in range(2):
                pso = ps_st.tile([KB, QC], F32, tag="st", name=f"pso{qb}_{oc}")
                for fc in range(2):
                    nc.tensor.matmul(
                        pso[:],
                        ats[fc][:, qb * KB : (qb + 1) * KB],
                        wo[fc][:, oc * QC : (oc + 1) * QC],
                        start=(fc == 0),
                        stop=(fc == 1),
                    )
                ost = ostage.tile([KB, QC], F32, tag="ost", name=f"os{qb}_{oc}")
                nc.vector.tensor_copy(ost[:], pso[:])
                nc.sync.dma_start(
                    out_d[qb * KB : (qb + 1) * KB, oc * QC : (oc + 1) * QC], ost[:]
                )


def _build():
    if "nc" in _CACHE:
        return _CACHE["nc"]
    nc = bacc.Bacc(
        "TRN2", target_bir_lowering=False, debug=False, num_devices=N_CORES
    )
    with tile.TileContext(nc) as tc:
        with ExitStack() as ctx:
            tc._emit_ctx = ctx
            _emit(tc)
    nc.compile()
    _CACHE["nc"] = nc
    return nc


def _numpy_fallback(q, attention_mask, Wq, Wk, Wv, Wo):
    import math

    b, s, _ = q.shape
    causal = np.tril(np.ones((s, s), bool))
    valid = attention_mask != 0
    mask = causal[None] & valid[:, :, None] & valid[:, None, :]
    mask = mask[:, None]
    out = np.zeros((b, s, H), np.float32)
    for bi in range(b):
        x = q[bi]
        nh = x.shape[1] // HD
        qh = (x @ Wq.T).reshape(s, nh, HD).transpose(1, 0, 2)
        kh = (x @ Wk.T).reshape(s, nh, HD).transpose(1, 0, 2)
        vh = (x @ Wv.T).reshape(s, nh, HD).transpose(1, 0, 2)
        sc = np.einsum("hqd,hkd->hqk", qh, kh) / math.sqrt(HD)
        sc = np.where(mask[bi], sc, np.float32(-1e6))
        sc = sc - sc.max(-1, keepdims=True)
        e = np.exp(sc)
        p = e / e.sum(-1, keepdims=True)
        p = np.where(mask[bi], p, np.float32(0.0))
        o = np.einsum("hqk,hkd->hqd", p, vh).transpose(1, 0, 2).reshape(s, -1)
        out[bi] = o @ Wo.T
    return out


def _run(q, attention_mask, Wq, Wk, Wv, Wo, trace=False, **trace_kwargs):
    q = np.ascontiguousarray(np.asarray(q, dtype=np.float32))
    Wq = np.asarray(Wq, dtype=np.float32)
    Wk = np.asarray(Wk, dtype=np.float32)
    Wv = np.asarray(Wv, dtype=np.float32)
    Wo = np.asarray(Wo, dtype=np.float32)
    am = np.asarray(attention_mask)
    if q.shape != (B, S, H) or not np.all(am != 0):
        return _numpy_fallback(q, am, Wq, Wk, Wv, Wo), None

    idx = np.arange(KB)
    mtri = (idx[:, None] <= idx[None, :]).astype(np.float32)

    in_maps = []
    for c in range(N_CORES):
        b, g = c // 4, c % 4
        fs = slice(F * g, F * (g + 1))
        in_maps.append(
            {
                "xT": np.ascontiguousarray(q[b].T),
                "wqT": np.ascontiguousarray(Wq[fs, :].T),
                "wkT": np.ascontiguousarray(Wk[fs, :].T),
                "wvT": np.ascontiguousarray(Wv[fs, :].T),
                "woT": np.ascontiguousarray(Wo[:, fs].T),
                "mtri": mtri,
            }
        )

    nc = _build()
    res = bass_utils.run_bass_kernel_spmd(
        nc, in_maps, core_ids=list(range(N_CORES)), trace=trace, **trace_kwargs
    )
    outs = [r["out"] for r in res.results]
    full = np.empty((B, S, H), np.float32)
    for b in range(B):
        full[b] = outs[4 * b] + outs[4 * b + 1] + outs[4 * b + 2] + outs[4 * b + 3]
    return full, res


def kernel(q, attention_mask, Wq, Wk, Wv, Wo):
    out, _ = _run(q, attention_mask, Wq, Wk, Wv, Wo)
    return out
